# revision 14
# baseline (speedup 1.0000x reference)
"""BiLSTM-CRF Trainium2 kernel (8-core SPMD, direction-split data parallel).

Sharding: cores 0-3 run the FORWARD LSTM for 8 sequences each; cores 4-7 run
the BACKWARD LSTM (on host-time-reversed inputs) for the same 8-sequence
shards.  Every core: input projection (emb @ k + b), 256-step LSTM recurrence,
and a partial dense projection (h_dir @ W_half).  Host combines the two
direction partials, applies selu, and runs the (tiny) CRF in numpy.

Device layout: gate dimension (4H=2048) lives on SBUF partitions as 16 m-tiles
of 128; batch (8) is the matmul free dim.  Matmul inputs fp16, PSUM/state f32.
"""

import numpy as np

P = 128
B = 8            # sequences per core
T = 256
D = 768
DC = D // P      # 6  k-chunks of input dim
H = 512
HC = H // P      # 4  k-chunks of hidden dim
G = 4 * H        # 2048 gate dim
M = G // P       # 16 m-tiles
NCLS = 25
NT = B * T       # tokens per core, t-major: col = t*B + b

_STATE = {}


def _build(n_steps=T):
    import concourse.mybir as mybir
    from concourse import bacc
    import concourse.tile as tile

    fp16 = mybir.dt.float16
    f32 = mybir.dt.float32
    AF = mybir.ActivationFunctionType
    nt = B * n_steps

    nc = bacc.Bacc("TRN2", target_bir_lowering=False, debug=False, num_devices=8)
    embT = nc.dram_tensor("embT", [P, DC, nt], fp16, kind="ExternalInput").ap()
    kT = nc.dram_tensor("kT", [P, DC, M, P], fp16, kind="ExternalInput").ap()
    rT = nc.dram_tensor("rT", [P, HC, M, P], fp16, kind="ExternalInput").ap()
    bias = nc.dram_tensor("bias", [P, M], f32, kind="ExternalInput").ap()
    dT = nc.dram_tensor("dT", [P, HC, NCLS], fp16, kind="ExternalInput").ap()
    db = nc.dram_tensor("db", [NCLS, 1], f32, kind="ExternalInput").ap()
    plogT = nc.dram_tensor("plogT", [NCLS, nt], f32, kind="ExternalOutput").ap()

    Q = 4
    SQ = n_steps // Q            # steps per quarter
    TOKQ = SQ * B                # tokens per quarter

    with tile.TileContext(nc) as tc:
        with tc.tile_pool(name="pers", bufs=1) as pers, \
             tc.tile_pool(name="embp", bufs=2) as ep, \
             tc.tile_pool(name="pp", bufs=2, space="PSUM") as pp:
            xz_q = [pers.tile([P, SQ, M, B], fp16, tag=f"xz{q}",
                              name=f"xz{q}") for q in range(Q)]
            r_sb = pers.tile([P, HC, M, P], fp16, tag="rsb")
            nc.sync.dma_start(out=r_sb, in_=rT)
            h_all = pers.tile([P, HC, n_steps, B], fp16, tag="hall")
            bias_sb = pers.tile([P, M], f32, tag="bias")
            nc.sync.dma_start(out=bias_sb, in_=bias)
            d_sb = pers.tile([P, HC, NCLS], fp16, tag="dsb")
            nc.sync.dma_start(out=d_sb, in_=dT)
            db_sb = pers.tile([NCLS, 1], f32, tag="dbsb")
            nc.sync.dma_start(out=db_sb, in_=db)
            k_sb = pers.tile([P, DC, M, P], fp16, tag="ksb")
            nc.sync.dma_start(out=k_sb, in_=kT)

            emb_tiles = {}

            def load_emb(q):
                emb_t = ep.tile([P, DC, TOKQ], fp16, tag="embt")
                nc.sync.dma_start(
                    out=emb_t, in_=embT[:, :, q * TOKQ:(q + 1) * TOKQ])
                emb_tiles[q] = emb_t

            def emit_proj_m(q, m):
                """One m-tile of the input projection for quarter q."""
                emb_t = emb_tiles[q]
                ps = pp.tile([P, TOKQ], f32, tag="psproj")
                for kc in range(DC):
                    nc.tensor.matmul(
                        ps,
                        k_sb[:, kc, m, :],
                        emb_t[:, kc, :],
                        start=(kc == 0),
                        stop=(kc == DC - 1),
                    )
                nc.vector.tensor_scalar_add(
                    xz_q[q][:, :, m, :],
                    ps.rearrange("p (t b) -> p t b", b=B),
                    bias_sb[:, m:m + 1],
                )

            # quarter 0 projection upfront; quarters 1..3 interleave below
            load_emb(0)
            for m in range(M):
                emit_proj_m(0, m)

            # ---------------- LSTM recurrence -----------------------------
            with tc.tile_pool(name="state", bufs=3) as st, \
                 tc.tile_pool(name="zp", bufs=4, space="PSUM") as zp:
                c_prev = None
                h_prev = None
                for t in range(n_steps):
                    q, lt = divmod(t, SQ)
                    if t == 0:
                        gates = st.tile([P, M * B], f32, tag="gates")
                        nc.vector.tensor_copy(
                            gates.rearrange("p (m b) -> p m b", b=B),
                            xz_q[0][:, 0, :, :])
                    else:
                        z = zp.tile([P, M * B], f32, tag="z")
                        for m in range(M):
                            for kc in range(HC):
                                nc.tensor.matmul(
                                    z[:, m * B:(m + 1) * B],
                                    r_sb[:, kc, m, :],
                                    h_prev[:, kc * B:(kc + 1) * B],
                                    start=(kc == 0),
                                    stop=(kc == HC - 1),
                                )
                        gates = st.tile([P, M * B], f32, tag="gates")
                        nc.vector.tensor_add(
                            gates.rearrange("p (m b) -> p m b", b=B),
                            z.rearrange("p (m b) -> p m b", b=B),
                            xz_q[q][:, lt, :, :])
                    # interleave next quarter's projection into PE gaps
                    if q < Q - 1:
                        if lt == 0:
                            load_emb(q + 1)
                        for m in range(lt * M // SQ, (lt + 1) * M // SQ):
                            emit_proj_m(q + 1, m)
                    # gate columns: i=[0,4B) f=[4B,8B) g=[8B,12B) o=[12B,16B)
                    acts = st.tile([P, M * B], f32, tag="acts")
                    nc.scalar.activation(acts[:, 0:8 * B], gates[:, 0:8 * B],
                                         AF.Sigmoid)
                    nc.scalar.activation(acts[:, 8 * B:12 * B],
                                         gates[:, 8 * B:12 * B], AF.Tanh)
                    nc.scalar.activation(acts[:, 12 * B:16 * B],
                                         gates[:, 12 * B:16 * B], AF.Sigmoid)
                    if t == 0:
                        t1 = st.tile([P, HC * B], f32, tag="t1")
                        nc.vector.tensor_mul(t1, acts[:, 0:4 * B],
                                             acts[:, 8 * B:12 * B])
                        c_t = t1
                    else:
                        t2 = st.tile([P, HC * B], f32, tag="t2")
                        nc.vector.tensor_mul(t2, acts[:, 4 * B:8 * B], c_prev)
                        t1 = st.tile([P, HC * B], f32, tag="t1")
                        nc.vector.tensor_mul(t1, acts[:, 0:4 * B],
                                             acts[:, 8 * B:12 * B])
                        c_t = st.tile([P, HC * B], f32, tag="c")
                        nc.vector.tensor_add(c_t, t1, t2)
                    tc_t = st.tile([P, HC * B], f32, tag="tc")
                    nc.scalar.activation(tc_t, c_t, AF.Tanh)
                    h_t = st.tile([P, HC * B], fp16, tag="h")
                    nc.vector.tensor_mul(h_t, acts[:, 12 * B:16 * B], tc_t)
                    nc.vector.tensor_copy(
                        h_all[:, :, t, :],
                        h_t.rearrange("p (c b) -> p c b", b=B))
                    c_prev = c_t
                    h_prev = h_t

            # ---------------- partial dense: plog = h @ W_half + db -------
            with tc.tile_pool(name="dp", bufs=2, space="PSUM") as dp, \
                 tc.tile_pool(name="po", bufs=2) as po:
                for q in range(Q):
                    ps = dp.tile([NCLS, TOKQ], f32, tag="psd")
                    for kc in range(HC):
                        nc.tensor.matmul(
                            ps,
                            d_sb[:, kc, :],
                            h_all[:, kc, q * SQ:(q + 1) * SQ, :],
                            start=(kc == 0),
                            stop=(kc == HC - 1),
                        )
                    out_sb = po.tile([NCLS, TOKQ], f32, tag="outsb")
                    nc.vector.tensor_scalar_add(out_sb, ps, db_sb)
                    nc.sync.dma_start(
                        out=plogT[:, q * TOKQ:(q + 1) * TOKQ], in_=out_sb)

    nc.compile()
    return nc


# ------------------------- host-side helpers -------------------------------

def _prep_core_inputs(x, k, r, b, d_half, d_bias):
    """x: [B,T',D] f32 (already direction-ordered). Returns in_map dict."""
    n_steps = x.shape[1]
    embT = np.ascontiguousarray(
        x.astype(np.float16).transpose(2, 1, 0).reshape(DC, P, n_steps * B)
        .transpose(1, 0, 2))
    kT = np.ascontiguousarray(
        k.astype(np.float16).reshape(DC, P, M, P).transpose(1, 0, 2, 3))
    rT = np.ascontiguousarray(
        r.astype(np.float16).reshape(HC, P, M, P).transpose(1, 0, 2, 3))
    bias = np.ascontiguousarray(b.astype(np.float32).reshape(M, P).T)
    dT = np.ascontiguousarray(
        d_half.astype(np.float16).reshape(HC, P, NCLS).transpose(1, 0, 2))
    db = d_bias.astype(np.float32).reshape(NCLS, 1)
    return {"embT": embT, "kT": kT, "rT": rT, "bias": bias, "dT": dT, "db": db}


def _plog_from_out(out, n_steps):
    """out: [NCLS, n_steps*B] -> [B, n_steps, NCLS]"""
    return out.reshape(NCLS, n_steps, B).transpose(2, 1, 0)


def _selu(x):
    scale = 1.0507009873554805
    alpha = 1.6732632423543772
    return (scale * np.where(x > 0, x, alpha * (np.exp(np.minimum(x, 0.0)) - 1.0))
            ).astype(np.float32)


def _crf_nll(logits, tags, seq_lens, trans):
    Bf, Tf, N = logits.shape
    logits = logits.astype(np.float64)
    trans = trans.astype(np.float64)
    t_idx = np.arange(Tf)
    mask = t_idx[None, :] < seq_lens[:, None]
    maskf = mask.astype(np.float64)
    unary = np.take_along_axis(logits, tags[..., None].astype(np.int64),
                               axis=-1)[..., 0]
    unary_score = (unary * maskf).sum(-1)
    binary = trans[tags[:, :-1], tags[:, 1:]]
    binary_score = (binary * maskf[:, 1:]).sum(-1)
    alpha = logits[:, 0].copy()
    expT = np.exp(trans)
    for t in range(1, Tf):
        m = alpha.max(-1, keepdims=True)
        new = np.log(np.exp(alpha - m) @ expT) + m + logits[:, t]
        alpha = np.where(mask[:, t][:, None], new, alpha)
    m = alpha.max(-1, keepdims=True)
    log_z = (m + np.log(np.exp(alpha - m).sum(-1, keepdims=True)))[:, 0]
    return -(unary_score + binary_score - log_z)


def _run_device(in_maps):
    from concourse import bass_utils
    from concourse.bass_interp import get_hw_module

    if "nc" not in _STATE:
        _STATE["nc"] = _build(T)
    nc = _STATE["nc"]
    old = nc.m
    nc.m = get_hw_module(nc.m)
    try:
        res = bass_utils.run_bass_kernel_spmd(nc, in_maps,
                                              core_ids=list(range(8)))
    finally:
        nc.m = old
    return res.results


def kernel(embeddings, targets, seq_lens, k_fwd, r_fwd, b_fwd,
           k_bwd, r_bwd, b_bwd, dense_w, dense_b, transitions):
    Bfull = embeddings.shape[0]
    n_shards = Bfull // B
    in_maps = []
    for c in range(4):
        x = embeddings[c * B:(c + 1) * B]
        in_maps.append(_prep_core_inputs(
            x, k_fwd, r_fwd, b_fwd, dense_w[:H], dense_b))
    zeros_db = np.zeros_like(dense_b)
    for c in range(4):
        x = embeddings[c * B:(c + 1) * B][:, ::-1]
        in_maps.append(_prep_core_inputs(
            x, k_bwd, r_bwd, b_bwd, dense_w[H:], zeros_db))

    results = _run_device(in_maps)

    pre = np.empty((Bfull, T, NCLS), np.float32)
    for c in range(4):
        pf = _plog_from_out(results[c]["plogT"], T)
        pb = _plog_from_out(results[c + 4]["plogT"], T)[:, ::-1]
        pre[c * B:(c + 1) * B] = pf + pb

    logits = _selu(pre)
    seq_lens_c = np.clip(seq_lens, 1, T)
    nll = _crf_nll(logits, targets, seq_lens_c, transitions)
    loss = np.float32(nll.mean())
    return loss, logits


# revision 15
# speedup vs baseline: 1.0003x; 1.0003x over previous
"""BiLSTM-CRF Trainium2 kernel (8-core SPMD, direction-split data parallel).

Sharding: cores 0-3 run the FORWARD LSTM for 8 sequences each; cores 4-7 run
the BACKWARD LSTM (on host-time-reversed inputs) for the same 8-sequence
shards.  Every core: input projection (emb @ k + b), 256-step LSTM recurrence,
and a partial dense projection (h_dir @ W_half).  Host combines the two
direction partials, applies selu, and runs the (tiny) CRF in numpy.

Device layout: gate dimension (4H=2048) lives on SBUF partitions as 16 m-tiles
of 128; batch (8) is the matmul free dim.  Matmul inputs fp16, PSUM/state f32.
"""

import numpy as np

P = 128
B = 8            # sequences per core
T = 256
D = 768
DC = D // P      # 6  k-chunks of input dim
H = 512
HC = H // P      # 4  k-chunks of hidden dim
G = 4 * H        # 2048 gate dim
M = G // P       # 16 m-tiles
NCLS = 25
NT = B * T       # tokens per core, t-major: col = t*B + b

_STATE = {}


def _build(n_steps=T):
    import concourse.mybir as mybir
    from concourse import bacc
    import concourse.tile as tile

    fp16 = mybir.dt.float16
    f32 = mybir.dt.float32
    AF = mybir.ActivationFunctionType
    nt = B * n_steps

    nc = bacc.Bacc("TRN2", target_bir_lowering=False, debug=False, num_devices=8)
    embT = nc.dram_tensor("embT", [P, DC, nt], fp16, kind="ExternalInput").ap()
    kT = nc.dram_tensor("kT", [P, DC, M, P], fp16, kind="ExternalInput").ap()
    rT = nc.dram_tensor("rT", [P, HC, M, P], fp16, kind="ExternalInput").ap()
    bias = nc.dram_tensor("bias", [P, M], f32, kind="ExternalInput").ap()
    dT = nc.dram_tensor("dT", [P, HC, NCLS], fp16, kind="ExternalInput").ap()
    db = nc.dram_tensor("db", [NCLS, 1], f32, kind="ExternalInput").ap()
    plogT = nc.dram_tensor("plogT", [NCLS, nt], f32, kind="ExternalOutput").ap()

    Q = 4
    SQ = n_steps // Q            # steps per quarter
    TOKQ = SQ * B                # tokens per quarter

    with tile.TileContext(nc) as tc:
        with tc.tile_pool(name="pers", bufs=1) as pers, \
             tc.tile_pool(name="embp", bufs=2) as ep, \
             tc.tile_pool(name="pp", bufs=2, space="PSUM") as pp:
            xz_q = [pers.tile([P, SQ, M, B], fp16, tag=f"xz{q}",
                              name=f"xz{q}") for q in range(Q)]
            r_sb = pers.tile([P, HC, M, P], fp16, tag="rsb")
            nc.sync.dma_start(out=r_sb, in_=rT)
            h_all = pers.tile([P, HC, n_steps, B], fp16, tag="hall")
            bias_sb = pers.tile([P, M], f32, tag="bias")
            nc.sync.dma_start(out=bias_sb, in_=bias)
            d_sb = pers.tile([P, HC, NCLS], fp16, tag="dsb")
            nc.sync.dma_start(out=d_sb, in_=dT)
            db_sb = pers.tile([NCLS, 1], f32, tag="dbsb")
            nc.sync.dma_start(out=db_sb, in_=db)
            k_sb = pers.tile([P, DC, M, P], fp16, tag="ksb")
            nc.sync.dma_start(out=k_sb, in_=kT)

            emb_tiles = {}

            def load_emb(q):
                emb_t = ep.tile([P, DC, TOKQ], fp16, tag="embt")
                nc.sync.dma_start(
                    out=emb_t, in_=embT[:, :, q * TOKQ:(q + 1) * TOKQ])
                emb_tiles[q] = emb_t

            def emit_proj_m(q, m):
                """One m-tile of the input projection for quarter q."""
                emb_t = emb_tiles[q]
                ps = pp.tile([P, TOKQ], f32, tag="psproj")
                for kc in range(DC):
                    nc.tensor.matmul(
                        ps,
                        k_sb[:, kc, m, :],
                        emb_t[:, kc, :],
                        start=(kc == 0),
                        stop=(kc == DC - 1),
                    )
                nc.vector.tensor_scalar_add(
                    xz_q[q][:, :, m, :],
                    ps.rearrange("p (t b) -> p t b", b=B),
                    bias_sb[:, m:m + 1],
                )

            # quarter 0 projection upfront; quarters 1..3 interleave below
            load_emb(0)
            for m in range(M):
                emit_proj_m(0, m)

            # ---------------- LSTM recurrence -----------------------------
            with tc.tile_pool(name="state", bufs=3) as st, \
                 tc.tile_pool(name="zp", bufs=4, space="PSUM") as zp:
                c_prev = None
                for t in range(n_steps):
                    q, lt = divmod(t, SQ)
                    if t == 0:
                        gates = st.tile([P, M * B], f32, tag="gates")
                        nc.vector.tensor_copy(
                            gates.rearrange("p (m b) -> p m b", b=B),
                            xz_q[0][:, 0, :, :])
                    else:
                        z = zp.tile([P, M * B], f32, tag="z")
                        for m in range(M):
                            for kc in range(HC):
                                nc.tensor.matmul(
                                    z[:, m * B:(m + 1) * B],
                                    r_sb[:, kc, m, :],
                                    h_all[:, kc, t - 1, :],
                                    start=(kc == 0),
                                    stop=(kc == HC - 1),
                                )
                        gates = st.tile([P, M * B], f32, tag="gates")
                        nc.vector.tensor_add(
                            gates.rearrange("p (m b) -> p m b", b=B),
                            z.rearrange("p (m b) -> p m b", b=B),
                            xz_q[q][:, lt, :, :])
                    # interleave next quarter's projection into PE gaps
                    if q < Q - 1:
                        if lt == 0:
                            load_emb(q + 1)
                        for m in range(lt * M // SQ, (lt + 1) * M // SQ):
                            emit_proj_m(q + 1, m)
                    # gate columns: i=[0,4B) f=[4B,8B) g=[8B,12B) o=[12B,16B)
                    acts = st.tile([P, M * B], f32, tag="acts")
                    nc.scalar.activation(acts[:, 0:8 * B], gates[:, 0:8 * B],
                                         AF.Sigmoid)
                    nc.scalar.activation(acts[:, 8 * B:12 * B],
                                         gates[:, 8 * B:12 * B], AF.Tanh)
                    nc.scalar.activation(acts[:, 12 * B:16 * B],
                                         gates[:, 12 * B:16 * B], AF.Sigmoid)
                    if t == 0:
                        t1 = st.tile([P, HC * B], f32, tag="t1")
                        nc.vector.tensor_mul(t1, acts[:, 0:4 * B],
                                             acts[:, 8 * B:12 * B])
                        c_t = t1
                    else:
                        t2 = st.tile([P, HC * B], f32, tag="t2")
                        nc.vector.tensor_mul(t2, acts[:, 4 * B:8 * B], c_prev)
                        t1 = st.tile([P, HC * B], f32, tag="t1")
                        nc.vector.tensor_mul(t1, acts[:, 0:4 * B],
                                             acts[:, 8 * B:12 * B])
                        c_t = st.tile([P, HC * B], f32, tag="c")
                        nc.vector.tensor_add(c_t, t1, t2)
                    tc_t = st.tile([P, HC * B], f32, tag="tc")
                    nc.scalar.activation(tc_t, c_t, AF.Tanh)
                    # h written straight into h_all's step slice; next
                    # step's matmuls read it from there (no copy op)
                    nc.vector.tensor_mul(
                        h_all[:, :, t, :],
                        acts[:, 12 * B:16 * B].rearrange("p (c b) -> p c b",
                                                         b=B),
                        tc_t.rearrange("p (c b) -> p c b", b=B))
                    c_prev = c_t

            # ---------------- partial dense: plog = h @ W_half + db -------
            with tc.tile_pool(name="dp", bufs=2, space="PSUM") as dp, \
                 tc.tile_pool(name="po", bufs=2) as po:
                for q in range(Q):
                    ps = dp.tile([NCLS, TOKQ], f32, tag="psd")
                    for kc in range(HC):
                        nc.tensor.matmul(
                            ps,
                            d_sb[:, kc, :],
                            h_all[:, kc, q * SQ:(q + 1) * SQ, :],
                            start=(kc == 0),
                            stop=(kc == HC - 1),
                        )
                    out_sb = po.tile([NCLS, TOKQ], f32, tag="outsb")
                    nc.vector.tensor_scalar_add(out_sb, ps, db_sb)
                    nc.sync.dma_start(
                        out=plogT[:, q * TOKQ:(q + 1) * TOKQ], in_=out_sb)

    nc.compile()
    return nc


# ------------------------- host-side helpers -------------------------------

def _prep_core_inputs(x, k, r, b, d_half, d_bias):
    """x: [B,T',D] f32 (already direction-ordered). Returns in_map dict."""
    n_steps = x.shape[1]
    embT = np.ascontiguousarray(
        x.astype(np.float16).transpose(2, 1, 0).reshape(DC, P, n_steps * B)
        .transpose(1, 0, 2))
    kT = np.ascontiguousarray(
        k.astype(np.float16).reshape(DC, P, M, P).transpose(1, 0, 2, 3))
    rT = np.ascontiguousarray(
        r.astype(np.float16).reshape(HC, P, M, P).transpose(1, 0, 2, 3))
    bias = np.ascontiguousarray(b.astype(np.float32).reshape(M, P).T)
    dT = np.ascontiguousarray(
        d_half.astype(np.float16).reshape(HC, P, NCLS).transpose(1, 0, 2))
    db = d_bias.astype(np.float32).reshape(NCLS, 1)
    return {"embT": embT, "kT": kT, "rT": rT, "bias": bias, "dT": dT, "db": db}


def _plog_from_out(out, n_steps):
    """out: [NCLS, n_steps*B] -> [B, n_steps, NCLS]"""
    return out.reshape(NCLS, n_steps, B).transpose(2, 1, 0)


def _selu(x):
    scale = 1.0507009873554805
    alpha = 1.6732632423543772
    return (scale * np.where(x > 0, x, alpha * (np.exp(np.minimum(x, 0.0)) - 1.0))
            ).astype(np.float32)


def _crf_nll(logits, tags, seq_lens, trans):
    Bf, Tf, N = logits.shape
    logits = logits.astype(np.float64)
    trans = trans.astype(np.float64)
    t_idx = np.arange(Tf)
    mask = t_idx[None, :] < seq_lens[:, None]
    maskf = mask.astype(np.float64)
    unary = np.take_along_axis(logits, tags[..., None].astype(np.int64),
                               axis=-1)[..., 0]
    unary_score = (unary * maskf).sum(-1)
    binary = trans[tags[:, :-1], tags[:, 1:]]
    binary_score = (binary * maskf[:, 1:]).sum(-1)
    alpha = logits[:, 0].copy()
    expT = np.exp(trans)
    for t in range(1, Tf):
        m = alpha.max(-1, keepdims=True)
        new = np.log(np.exp(alpha - m) @ expT) + m + logits[:, t]
        alpha = np.where(mask[:, t][:, None], new, alpha)
    m = alpha.max(-1, keepdims=True)
    log_z = (m + np.log(np.exp(alpha - m).sum(-1, keepdims=True)))[:, 0]
    return -(unary_score + binary_score - log_z)


def _run_device(in_maps):
    from concourse import bass_utils
    from concourse.bass_interp import get_hw_module

    if "nc" not in _STATE:
        _STATE["nc"] = _build(T)
    nc = _STATE["nc"]
    old = nc.m
    nc.m = get_hw_module(nc.m)
    try:
        res = bass_utils.run_bass_kernel_spmd(nc, in_maps,
                                              core_ids=list(range(8)))
    finally:
        nc.m = old
    return res.results


def kernel(embeddings, targets, seq_lens, k_fwd, r_fwd, b_fwd,
           k_bwd, r_bwd, b_bwd, dense_w, dense_b, transitions):
    Bfull = embeddings.shape[0]
    n_shards = Bfull // B
    in_maps = []
    for c in range(4):
        x = embeddings[c * B:(c + 1) * B]
        in_maps.append(_prep_core_inputs(
            x, k_fwd, r_fwd, b_fwd, dense_w[:H], dense_b))
    zeros_db = np.zeros_like(dense_b)
    for c in range(4):
        x = embeddings[c * B:(c + 1) * B][:, ::-1]
        in_maps.append(_prep_core_inputs(
            x, k_bwd, r_bwd, b_bwd, dense_w[H:], zeros_db))

    results = _run_device(in_maps)

    pre = np.empty((Bfull, T, NCLS), np.float32)
    for c in range(4):
        pf = _plog_from_out(results[c]["plogT"], T)
        pb = _plog_from_out(results[c + 4]["plogT"], T)[:, ::-1]
        pre[c * B:(c + 1) * B] = pf + pb

    logits = _selu(pre)
    seq_lens_c = np.clip(seq_lens, 1, T)
    nll = _crf_nll(logits, targets, seq_lens_c, transitions)
    loss = np.float32(nll.mean())
    return loss, logits


# revision 16
# speedup vs baseline: 1.0031x; 1.0028x over previous
"""BiLSTM-CRF Trainium2 kernel (8-core SPMD, direction-split data parallel).

Sharding: cores 0-3 run the FORWARD LSTM for 8 sequences each; cores 4-7 run
the BACKWARD LSTM (on host-time-reversed inputs) for the same 8-sequence
shards.  Every core: input projection (emb @ k + b), 256-step LSTM recurrence,
and a partial dense projection (h_dir @ W_half).  Host combines the two
direction partials, applies selu, and runs the (tiny) CRF in numpy.

Device layout: gate dimension (4H=2048) lives on SBUF partitions as 16 m-tiles
of 128; batch (8) is the matmul free dim.  Matmul inputs fp16, PSUM/state f32.
"""

import numpy as np

P = 128
B = 8            # sequences per core
T = 256
D = 768
DC = D // P      # 6  k-chunks of input dim
H = 512
HC = H // P      # 4  k-chunks of hidden dim
G = 4 * H        # 2048 gate dim
M = G // P       # 16 m-tiles
NCLS = 25
NT = B * T       # tokens per core, t-major: col = t*B + b

_STATE = {}


def _build(n_steps=T):
    import concourse.mybir as mybir
    from concourse import bacc
    import concourse.tile as tile

    fp16 = mybir.dt.float16
    f32 = mybir.dt.float32
    AF = mybir.ActivationFunctionType
    nt = B * n_steps

    nc = bacc.Bacc("TRN2", target_bir_lowering=False, debug=False, num_devices=8)
    embT = nc.dram_tensor("embT", [P, DC, nt], fp16, kind="ExternalInput").ap()
    kT = nc.dram_tensor("kT", [P, DC, M, P], fp16, kind="ExternalInput").ap()
    rT = nc.dram_tensor("rT", [P, HC, M, P], fp16, kind="ExternalInput").ap()
    bias = nc.dram_tensor("bias", [P, M], f32, kind="ExternalInput").ap()
    dT = nc.dram_tensor("dT", [P, HC, NCLS], fp16, kind="ExternalInput").ap()
    db = nc.dram_tensor("db", [NCLS, 1], f32, kind="ExternalInput").ap()
    plogT = nc.dram_tensor("plogT", [NCLS, nt], f32, kind="ExternalOutput").ap()

    Q = 4
    SQ = n_steps // Q            # steps per quarter
    TOKQ = SQ * B                # tokens per quarter

    with tile.TileContext(nc) as tc:
        with tc.tile_pool(name="pers", bufs=1) as pers, \
             tc.tile_pool(name="embp", bufs=2) as ep, \
             tc.tile_pool(name="pp", bufs=2, space="PSUM") as pp:
            xz_q = [pers.tile([P, SQ, M, B], fp16, tag=f"xz{q}",
                              name=f"xz{q}") for q in range(Q)]
            r_sb = pers.tile([P, HC, M, P], fp16, tag="rsb")
            nc.sync.dma_start(out=r_sb, in_=rT)
            h_all = pers.tile([P, HC, n_steps, B], fp16, tag="hall")
            bias_sb = pers.tile([P, M], f32, tag="bias")
            nc.sync.dma_start(out=bias_sb, in_=bias)
            d_sb = pers.tile([P, HC, NCLS], fp16, tag="dsb")
            nc.sync.dma_start(out=d_sb, in_=dT)
            db_sb = pers.tile([NCLS, 1], f32, tag="dbsb")
            nc.sync.dma_start(out=db_sb, in_=db)
            k_sb = pers.tile([P, DC, M, P], fp16, tag="ksb")
            nc.sync.dma_start(out=k_sb, in_=kT)

            emb_tiles = {}

            def load_emb(q):
                emb_t = ep.tile([P, DC, TOKQ], fp16, tag="embt")
                nc.sync.dma_start(
                    out=emb_t, in_=embT[:, :, q * TOKQ:(q + 1) * TOKQ])
                emb_tiles[q] = emb_t

            def emit_proj_m(q, m):
                """One m-tile of the input projection for quarter q."""
                emb_t = emb_tiles[q]
                ps = pp.tile([P, TOKQ], f32, tag="psproj")
                for kc in range(DC):
                    nc.tensor.matmul(
                        ps,
                        k_sb[:, kc, m, :],
                        emb_t[:, kc, :],
                        start=(kc == 0),
                        stop=(kc == DC - 1),
                    )
                nc.vector.tensor_scalar_add(
                    xz_q[q][:, :, m, :],
                    ps.rearrange("p (t b) -> p t b", b=B),
                    bias_sb[:, m:m + 1],
                )

            # quarter 0 projection upfront; quarters 1..3 interleave below
            load_emb(0)
            for m in range(M):
                emit_proj_m(0, m)


            def emit_dense(qd, dp, po):
                ps = dp.tile([NCLS, TOKQ], f32, tag="psd", name="psd")
                for kc in range(HC):
                    nc.tensor.matmul(
                        ps, d_sb[:, kc, :],
                        h_all[:, kc, qd * SQ:(qd + 1) * SQ, :],
                        start=(kc == 0), stop=(kc == HC - 1))
                out_sb = po.tile([NCLS, TOKQ], f32, tag="outsb", name="outsb")
                nc.vector.tensor_scalar_add(out_sb, ps, db_sb)
                nc.sync.dma_start(
                    out=plogT[:, qd * TOKQ:(qd + 1) * TOKQ], in_=out_sb)
            # ---------------- LSTM recurrence -----------------------------
            with tc.tile_pool(name="state", bufs=3) as st, \
                 tc.tile_pool(name="zp", bufs=2, space="PSUM") as zp, \
                 tc.tile_pool(name="dpi", bufs=2, space="PSUM") as dpi, \
                 tc.tile_pool(name="poi", bufs=2) as poi:
                c_prev = None
                for t in range(n_steps):
                    q, lt = divmod(t, SQ)
                    if t == 0:
                        gates = st.tile([P, M * B], f32, tag="gates")
                        nc.vector.tensor_copy(
                            gates.rearrange("p (m b) -> p m b", b=B),
                            xz_q[0][:, 0, :, :])
                    else:
                        z = zp.tile([P, M * B], f32, tag="z")
                        for m in range(M):
                            for kc in range(HC):
                                nc.tensor.matmul(
                                    z[:, m * B:(m + 1) * B],
                                    r_sb[:, kc, m, :],
                                    h_all[:, kc, t - 1, :],
                                    start=(kc == 0),
                                    stop=(kc == HC - 1),
                                )
                        gates = st.tile([P, M * B], f32, tag="gates")
                        nc.vector.tensor_add(
                            gates.rearrange("p (m b) -> p m b", b=B),
                            z.rearrange("p (m b) -> p m b", b=B),
                            xz_q[q][:, lt, :, :])
                    # interleave next quarter's projection into PE gaps
                    if q < Q - 1:
                        if lt == 0:
                            load_emb(q + 1)
                            if q > 0:
                                emit_dense(q - 1, dpi, poi)
                        for m in range(lt * M // SQ, (lt + 1) * M // SQ):
                            emit_proj_m(q + 1, m)
                    # gate columns: i=[0,4B) f=[4B,8B) g=[8B,12B) o=[12B,16B)
                    acts = st.tile([P, M * B], f32, tag="acts")
                    nc.scalar.activation(acts[:, 0:8 * B], gates[:, 0:8 * B],
                                         AF.Sigmoid)
                    nc.scalar.activation(acts[:, 8 * B:12 * B],
                                         gates[:, 8 * B:12 * B], AF.Tanh)
                    nc.scalar.activation(acts[:, 12 * B:16 * B],
                                         gates[:, 12 * B:16 * B], AF.Sigmoid)
                    if t == 0:
                        t1 = st.tile([P, HC * B], f32, tag="t1")
                        nc.vector.tensor_mul(t1, acts[:, 0:4 * B],
                                             acts[:, 8 * B:12 * B])
                        c_t = t1
                    else:
                        t2 = st.tile([P, HC * B], f32, tag="t2")
                        nc.vector.tensor_mul(t2, acts[:, 4 * B:8 * B], c_prev)
                        t1 = st.tile([P, HC * B], f32, tag="t1")
                        nc.vector.tensor_mul(t1, acts[:, 0:4 * B],
                                             acts[:, 8 * B:12 * B])
                        c_t = st.tile([P, HC * B], f32, tag="c")
                        nc.vector.tensor_add(c_t, t1, t2)
                    tc_t = st.tile([P, HC * B], f32, tag="tc")
                    nc.scalar.activation(tc_t, c_t, AF.Tanh)
                    # h written straight into h_all's step slice; next
                    # step's matmuls read it from there (no copy op)
                    nc.vector.tensor_mul(
                        h_all[:, :, t, :],
                        acts[:, 12 * B:16 * B].rearrange("p (c b) -> p c b",
                                                         b=B),
                        tc_t.rearrange("p (c b) -> p c b", b=B))
                    c_prev = c_t

            # tail: last two quarters' dense
            with tc.tile_pool(name="dp", bufs=2, space="PSUM") as dp, \
                 tc.tile_pool(name="po", bufs=2) as po:
                for qd in (Q - 2, Q - 1):
                    ps = dp.tile([NCLS, TOKQ], f32, tag="psd")
                    for kc in range(HC):
                        nc.tensor.matmul(
                            ps, d_sb[:, kc, :],
                            h_all[:, kc, qd * SQ:(qd + 1) * SQ, :],
                            start=(kc == 0), stop=(kc == HC - 1))
                    out_sb = po.tile([NCLS, TOKQ], f32, tag="outsb")
                    nc.vector.tensor_scalar_add(out_sb, ps, db_sb)
                    nc.sync.dma_start(
                        out=plogT[:, qd * TOKQ:(qd + 1) * TOKQ], in_=out_sb)

    nc.compile()
    return nc


# ------------------------- host-side helpers -------------------------------

def _prep_core_inputs(x, k, r, b, d_half, d_bias):
    """x: [B,T',D] f32 (already direction-ordered). Returns in_map dict."""
    n_steps = x.shape[1]
    embT = np.ascontiguousarray(
        x.astype(np.float16).transpose(2, 1, 0).reshape(DC, P, n_steps * B)
        .transpose(1, 0, 2))
    kT = np.ascontiguousarray(
        k.astype(np.float16).reshape(DC, P, M, P).transpose(1, 0, 2, 3))
    rT = np.ascontiguousarray(
        r.astype(np.float16).reshape(HC, P, M, P).transpose(1, 0, 2, 3))
    bias = np.ascontiguousarray(b.astype(np.float32).reshape(M, P).T)
    dT = np.ascontiguousarray(
        d_half.astype(np.float16).reshape(HC, P, NCLS).transpose(1, 0, 2))
    db = d_bias.astype(np.float32).reshape(NCLS, 1)
    return {"embT": embT, "kT": kT, "rT": rT, "bias": bias, "dT": dT, "db": db}


def _plog_from_out(out, n_steps):
    """out: [NCLS, n_steps*B] -> [B, n_steps, NCLS]"""
    return out.reshape(NCLS, n_steps, B).transpose(2, 1, 0)


def _selu(x):
    scale = 1.0507009873554805
    alpha = 1.6732632423543772
    return (scale * np.where(x > 0, x, alpha * (np.exp(np.minimum(x, 0.0)) - 1.0))
            ).astype(np.float32)


def _crf_nll(logits, tags, seq_lens, trans):
    Bf, Tf, N = logits.shape
    logits = logits.astype(np.float64)
    trans = trans.astype(np.float64)
    t_idx = np.arange(Tf)
    mask = t_idx[None, :] < seq_lens[:, None]
    maskf = mask.astype(np.float64)
    unary = np.take_along_axis(logits, tags[..., None].astype(np.int64),
                               axis=-1)[..., 0]
    unary_score = (unary * maskf).sum(-1)
    binary = trans[tags[:, :-1], tags[:, 1:]]
    binary_score = (binary * maskf[:, 1:]).sum(-1)
    alpha = logits[:, 0].copy()
    expT = np.exp(trans)
    for t in range(1, Tf):
        m = alpha.max(-1, keepdims=True)
        new = np.log(np.exp(alpha - m) @ expT) + m + logits[:, t]
        alpha = np.where(mask[:, t][:, None], new, alpha)
    m = alpha.max(-1, keepdims=True)
    log_z = (m + np.log(np.exp(alpha - m).sum(-1, keepdims=True)))[:, 0]
    return -(unary_score + binary_score - log_z)


def _run_device(in_maps):
    from concourse import bass_utils
    from concourse.bass_interp import get_hw_module

    if "nc" not in _STATE:
        _STATE["nc"] = _build(T)
    nc = _STATE["nc"]
    old = nc.m
    nc.m = get_hw_module(nc.m)
    try:
        res = bass_utils.run_bass_kernel_spmd(nc, in_maps,
                                              core_ids=list(range(8)))
    finally:
        nc.m = old
    return res.results


def kernel(embeddings, targets, seq_lens, k_fwd, r_fwd, b_fwd,
           k_bwd, r_bwd, b_bwd, dense_w, dense_b, transitions):
    Bfull = embeddings.shape[0]
    n_shards = Bfull // B
    in_maps = []
    for c in range(4):
        x = embeddings[c * B:(c + 1) * B]
        in_maps.append(_prep_core_inputs(
            x, k_fwd, r_fwd, b_fwd, dense_w[:H], dense_b))
    zeros_db = np.zeros_like(dense_b)
    for c in range(4):
        x = embeddings[c * B:(c + 1) * B][:, ::-1]
        in_maps.append(_prep_core_inputs(
            x, k_bwd, r_bwd, b_bwd, dense_w[H:], zeros_db))

    results = _run_device(in_maps)

    pre = np.empty((Bfull, T, NCLS), np.float32)
    for c in range(4):
        pf = _plog_from_out(results[c]["plogT"], T)
        pb = _plog_from_out(results[c + 4]["plogT"], T)[:, ::-1]
        pre[c * B:(c + 1) * B] = pf + pb

    logits = _selu(pre)
    seq_lens_c = np.clip(seq_lens, 1, T)
    nll = _crf_nll(logits, targets, seq_lens_c, transitions)
    loss = np.float32(nll.mean())
    return loss, logits
